# revision 13
# baseline (speedup 1.0000x reference)
"""BiLSTM Trainium2 kernel v4: dual-window interleaved scan + column-tiled PE.

Sharding (8 cores, SPMD): sequence-split; core s owns output steps
[64s, 64s+64) for BOTH directions, processed as TWO interleaved 32-step
windows (A: [64s, 64s+32), B: [64s+32, 64s+64)), each preceded by a KW-step
warmup that converges to the true state through LSTM forget-gate decay.
Interleaving two independent recurrences hides each window's serial
activation/cell-update chain under the other window's matmuls.

Per-iteration j (L = KW + 32 iterations), per window, folded layout
[128, 256] (rows 0:64 = state rows x gate-col half A, 64:128 = half B;
state rows = 32 fwd + 32 bwd):
  gates:  two PSUM tiles [128, 2, 256] hold (f,i) and (o,ch).  Recurrent
      matmuls run COLUMN-TILED: tile T0 (psum partitions 0:64) and T1
      (64:128) execute concurrently in the PE array with the same 64-wide
      h.T stationary and different whT column halves -- 2x matmul
      throughput vs the 128-wide zero-padded fold.  PSUM is prefilled with
      the x-projection by the Pool engine (one [128,2,256] copy per pair).
  acts:   one fused sigmoid over (f,i), tanh(ch), sigmoid(o) on ACT.
  cell:   cm=f*c (Pool), ic=i*ch (DVE), c=cm+ic (DVE), tanh(c) (ACT),
      h=o*tanh(c) (DVE, bf16).
  h.T:    4 column-tiled regular matmuls against a bf16 identity (f32
      PSUM) -> one DVE copy into the bf16 staging block stg[128,2,16,128]
      (cols 0:64 = k-chunks 0/1, 64:128 = chunks 2/3).
  xp:     staged in DRAM bf16 (host-swizzled (half,gate,256) columns) by
      interleaved xproj emission blocks; consumed through [128,4,1024]
      SBUF run buffers loaded 4 iterations at a time (fwd ascending, bwd
      descending via negative-stride DMA), one DMA per quadrant.
  outproj: at each 16-iteration emission block boundary, out.T[128, 512]
      per direction is computed straight from the stg SBUF block (no DRAM
      round-trip) and DMA'd out.
Boundary exactness (fwd t=0 zeros, bwd t=511 learned init) restored by a
masked state merge at the end of warmup, as in v3.
"""

import sys

sys.path.insert(0, "/opt/trn_rl_repo")

import numpy as np
from contextlib import ExitStack

from concourse import bass, bacc, tile, mybir
from concourse.bass_utils import run_bass_kernel_spmd

F32 = mybir.dt.float32
F32R = mybir.dt.float32r
BF16 = mybir.dt.bfloat16
AF = mybir.ActivationFunctionType

B, T, I, H, O = 32, 512, 256, 512, 128
G = 4 * H
NCORES = 8
NWIN = 2
NSW = T // (NCORES * NWIN)      # 32 output steps per window
KW = 16                         # warmup steps
L = KW + NSW                    # 48 scan iterations
WIN = 2 * NSW + 2 * KW          # 96 xp positions per core (window union)
RUN = 4                         # xp run-buffer length (iterations)
PROBE_CONST_LHS = False
NBLK = (WIN * B) // 128         # 24 xproj emission blocks
ROWS_O = NWIN * NSW * B         # 2048 output rows per core


def _r(ap):
    return ap.bitcast(F32R)


def _emission_plan():
    """Block emission order grouped by earliest-need run e(b), and the
    per-iteration schedule: groups 0,1 pre-loop; group g>=2 during
    iterations [4(g-2), 4(g-1))."""
    nrun = L // RUN
    e = {}
    for b_ in range(NBLK):
        cands = []
        if b_ < 12:
            cands.append(b_)           # win A fwd, run b
        if 4 <= b_ <= 15:
            cands.append(15 - b_)      # win A bwd
        if 8 <= b_ <= 19:
            cands.append(b_ - 8)       # win B fwd
        if 12 <= b_ <= 23:
            cands.append(23 - b_)      # win B bwd
        cands = [c for c in cands if 0 <= c < nrun]
        e[b_] = min(cands)
    groups = {}
    for b_, g in e.items():
        groups.setdefault(g, []).append(b_)
    pre = sorted(groups.get(0, [])) + sorted(groups.get(1, []))
    sched = {}  # iter -> list of blocks to emit
    for g in sorted(groups):
        if g < 2:
            continue
        blocks = sorted(groups[g])
        start = 4 * (g - 2)
        for i_, b_ in enumerate(blocks):
            sched.setdefault(start + i_, []).append(b_)
    return pre, sched


def build_program_v4(repeats=1):
    nc = bacc.Bacc(
        "TRN2", target_bir_lowering=False, debug=False, num_devices=NCORES
    )

    xt = nc.dram_tensor("xt", [I, WIN * B], F32, kind="ExternalInput").ap()
    wxT = nc.dram_tensor("wxT", [I, G], F32, kind="ExternalInput").ap()
    bxpad = nc.dram_tensor("bxpad", [128, G], F32, kind="ExternalInput").ap()
    bxfull = nc.dram_tensor("bxfull", [128, G], BF16, kind="ExternalInput").ap()
    whT = nc.dram_tensor("whT", [H, G], BF16, kind="ExternalInput").ap()
    e0 = nc.dram_tensor("e0", [128, 128], F32, kind="ExternalInput").ap()
    h0stg = nc.dram_tensor("h0stg", [128, 2, 128], F32, kind="ExternalInput").ap()
    c0fold = nc.dram_tensor("c0fold", [128, 256], F32, kind="ExternalInput").ap()
    mfull = nc.dram_tensor("mfull", [NWIN, 128, 256], F32, kind="ExternalInput").ap()
    cim = nc.dram_tensor("cim", [NWIN, 128, 256], F32, kind="ExternalInput").ap()
    him = nc.dram_tensor("him", [NWIN, 128, 256], F32, kind="ExternalInput").ap()
    wdTf = nc.dram_tensor("wdTf", [H, O], BF16, kind="ExternalInput").ap()
    wdTb = nc.dram_tensor("wdTb", [H, O], BF16, kind="ExternalInput").ap()
    ob = nc.dram_tensor("ob", [O, 1], F32, kind="ExternalInput").ap()
    ident = nc.dram_tensor("ident", [128, 128], F32, kind="ExternalInput").ap()
    outTf = nc.dram_tensor("outTf", [O, ROWS_O], F32, kind="ExternalOutput").ap()
    outTb = nc.dram_tensor("outTb", [O, ROWS_O], F32, kind="ExternalOutput").ap()

    xp_d = nc.dram_tensor("xp_d", [WIN, B, G], BF16, kind="Internal").ap()

    with tile.TileContext(nc) as tc, ExitStack() as ctx:
        const = ctx.enter_context(tc.tile_pool(name="const", bufs=1))
        gps_pool = ctx.enter_context(tc.tile_pool(name="gps", bufs=2, space="PSUM"))
        xo_pool = ctx.enter_context(tc.tile_pool(name="xo", bufs=2, space="PSUM"))
        pst_pool = ctx.enter_context(tc.tile_pool(name="pst", bufs=2, space="PSUM"))
        xst_pool = ctx.enter_context(tc.tile_pool(name="xst", bufs=3))
        xpf_pool = ctx.enter_context(tc.tile_pool(name="xpf", bufs=2))
        stg_pool = ctx.enter_context(tc.tile_pool(name="stg", bufs=2))
        act_pool = ctx.enter_context(tc.tile_pool(name="act", bufs=8))
        tmp_pool = ctx.enter_context(tc.tile_pool(name="tmp", bufs=3))
        xq_pool = ctx.enter_context(tc.tile_pool(name="xq", bufs=2))
        osb_pool = ctx.enter_context(tc.tile_pool(name="osb", bufs=2))

        # ---- constants ----
        wxT_sb = const.tile([128, 2, G], F32R)
        for c in range(2):
            nc.sync.dma_start(wxT_sb[:, c, :], _r(wxT[c * 128:(c + 1) * 128, :]))
        bxpad_sb = const.tile([128, G], F32R)
        nc.sync.dma_start(bxpad_sb[:], _r(bxpad[:]))
        e0_sb = const.tile([128, 128], F32R)
        nc.sync.dma_start(e0_sb[:], _r(e0[:]))
        bxf_sb = const.tile([128, G], BF16)
        nc.sync.dma_start(bxf_sb[:], bxfull[:])
        whT_sb = const.tile([128, 4, G], BF16)
        for c in range(4):
            nc.sync.dma_start(whT_sb[:, c, :], whT[c * 128:(c + 1) * 128, :])
        h0f = const.tile([128, 2, 128], F32)
        nc.sync.dma_start(h0f[:], h0stg[:])
        h0_sb = const.tile([128, 2, 128], BF16, name="h0ent_sb")
        nc.vector.tensor_copy(h0_sb[:], h0f[:])
        idf = const.tile([128, 128], F32)
        nc.sync.dma_start(idf[:], ident[:])
        id_sb = const.tile([128, 128], BF16)
        nc.vector.tensor_copy(id_sb[:], idf[:])
        wdT_sb = {}
        for d, src in (("f", wdTf), ("b", wdTb)):
            wdT_sb[d] = const.tile([128, 4, O], BF16, name=f"wdT{d}_sb")
            for c in range(4):
                nc.sync.dma_start(wdT_sb[d][:, c, :], src[c * 128:(c + 1) * 128, :])
        ob_sb = const.tile([O, 1], F32)
        nc.sync.dma_start(ob_sb[:], ob[:])
        zb = const.tile([128, 1], F32)
        nc.gpsimd.memset(zb[:], 0.0)
        mf_sb = const.tile([128, NWIN, 256], F32)
        nc.sync.dma_start(mf_sb[:], mfull.transpose([1, 0, 2]))
        cim_sb = const.tile([128, NWIN, 256], F32)
        nc.sync.dma_start(cim_sb[:], cim.transpose([1, 0, 2]))
        him_sb = const.tile([128, NWIN, 256], F32)
        nc.sync.dma_start(him_sb[:], him.transpose([1, 0, 2]))
        c_sb = [const.tile([128, 256], F32, name=f"c{w}_sb") for w in range(NWIN)]
        h_sb = [const.tile([128, 256], BF16, name=f"h{w}_sb") for w in range(NWIN)]
        for w in range(NWIN):
            nc.gpsimd.memset(h_sb[w][:], 0.0)

        for rep in range(repeats):
            _phases_v4(
                nc, tc, rep, xt, xp_d, outTf, outTb, c0fold,
                wxT_sb, bxpad_sb, bxf_sb, e0_sb, whT_sb, h0_sb, id_sb, wdT_sb, ob_sb,
                zb, mf_sb, cim_sb, him_sb, c_sb, h_sb,
                gps_pool, xo_pool, pst_pool, xst_pool, xpf_pool, stg_pool,
                act_pool, tmp_pool, xq_pool, osb_pool,
            )

    nc.compile()
    return nc


def _phases_v4(
    nc, tc, rep, xt, xp_d, outTf, outTb, c0fold,
    wxT_sb, bxpad_sb, bxf_sb, e0_sb, whT_sb, h0_sb, id_sb, wdT_sb, ob_sb,
    zb, mf_sb, cim_sb, him_sb, c_sb, h_sb,
    gps_pool, xo_pool, pst_pool, xst_pool, xpf_pool, stg_pool,
    act_pool, tmp_pool, xq_pool, osb_pool,
):
    # per-repeat state init (fwd rows zero, bwd rows learned cell init)
    for w in range(NWIN):
        nc.sync.dma_start(c_sb[w][:], c0fold[:])

    def emit_xproj(b_):
        """xproj emission for block b_ (positions 4b..4b+3): 4 column-tiled
        psums [128, 512] -> one bf16 xq tile -> one DMA into xp_d."""
        xst = xst_pool.tile([128, 2, 128], F32R, tag="xst", name=f"xst{rep}_{b_}")
        for c in range(2):
            nc.sync.dma_start(
                xst[:, c, :],
                _r(xt[c * 128:(c + 1) * 128, b_ * 128:(b_ + 1) * 128]),
            )
        xq = xq_pool.tile([128, G], BF16, tag="xq", name=f"xq{rep}_{b_}")
        for s in range(4):
            # f32r matmuls cannot be column-tiled (invalid ISA); these run
            # untiled (128x128 mode) -- the PE drains on the mode switch.
            # Bias: s<2 folded into the DVE evacuation add (bxfull), s>=2
            # via the e0 ones-row matmul + ACT copy evacuation.
            ps = xo_pool.tile([128, 512], F32, tag="xo", name=f"xps{rep}_{b_}_{s}")
            for c in range(2):
                nc.tensor.matmul(
                    ps[:],
                    xst[:, c, :],
                    wxT_sb[:, c, s * 512:(s + 1) * 512],
                    start=(c == 0), stop=False,
                )
            nc.tensor.matmul(
                ps[:],
                e0_sb[:],
                bxpad_sb[:, s * 512:(s + 1) * 512],
                start=False, stop=True,
            )
            if s < 2:
                nc.vector.tensor_copy(xq[:, s * 512:(s + 1) * 512], ps[:])
            else:
                nc.scalar.activation(
                    xq[:, s * 512:(s + 1) * 512], ps[:], AF.Copy
                )
        nc.sync.dma_start(
            xp_d.flatten_outer_dims()[b_ * 128:(b_ + 1) * 128, :], xq[:]
        )

    def issue_run(w, m):
        """Load xp run m for window w into an xpf tile: fwd positions
        32w+4m.. ascending on partitions 0:32/64:96, bwd positions
        63+32w-4m.. descending on 32:64/96:128."""
        xpf = xpf_pool.tile([128, RUN, 1024], BF16, tag=f"xpf{w}",
                            name=f"xpf{rep}_{w}_{m}")
        fb = 32 * w + RUN * m
        bb = 63 + 32 * w - RUN * m - (RUN - 1)
        nc.sync.dma_start(
            xpf[0:32, :, :], xp_d[fb:fb + RUN, :, 0:1024].transpose([1, 0, 2])
        )
        nc.sync.dma_start(
            xpf[64:96, :, :],
            xp_d[fb:fb + RUN, :, 1024:2048].transpose([1, 0, 2]),
        )
        nc.sync.dma_start(
            xpf[32:64, :, :],
            xp_d[bb:bb + RUN, :, 0:1024][::-1].transpose([1, 0, 2]),
        )
        nc.sync.dma_start(
            xpf[96:128, :, :],
            xp_d[bb:bb + RUN, :, 1024:2048][::-1].transpose([1, 0, 2]),
        )
        return xpf

    pre, sched = _emission_plan()
    for b_ in pre:
        emit_xproj(b_)

    xpf_cur = [issue_run(w, 0) for w in range(NWIN)]
    xpf_nxt = [issue_run(w, 1) for w in range(NWIN)]

    def outproj(w, blk, stgw):
        """Output projection for emission block blk of window w, straight
        from the stg SBUF block (fwd rows = cols 0:32/64:96 per t2)."""
        for d, outdst, bias in (("f", outTf, ob_sb[0:O, 0:1]),
                                ("b", outTb, zb[0:O, 0:1])):
            ro = 32 if d == "b" else 0
            ps = xo_pool.tile([128, 512], F32, tag="xo",
                              name=f"ops{rep}_{w}_{blk}{d}")
            for k in range(4):
                rhs = (stgw[:, k, :, ro:ro + 32] if k < 2
                       else stgw[:, k - 2, :, 64 + ro:96 + ro])
                for tp in (0, 64):
                    nc.tensor.matmul(
                        ps[tp:tp + 64, :],
                        wdT_sb[d][:, k, tp:tp + 64],
                        rhs,
                        start=(k == 0), stop=(k == 3),
                        tile_position=(0, tp),
                    )
            osb = osb_pool.tile([O, 512], F32, tag="osb",
                                name=f"osb{rep}_{w}_{blk}{d}")
            nc.scalar.activation(osb[:], ps[:], AF.Identity, bias=bias)
            nc.sync.dma_start(
                outdst[:, w * 1024 + blk * 512:w * 1024 + (blk + 1) * 512],
                osb[:],
            )

    def emit_pst_stg(w, jp, stg_tile):
        """h(jp).T via 4 column-tiled regular matmuls (f32 psum) + 1 DVE copy
        into stg slot jp%16.  Issued at the TOP of the window's next
        iteration block so it never head-blocks the other window's ready
        matmuls in the in-order PE queue."""
        pst = pst_pool.tile([128, 2, 128], F32, tag="pst",
                            name=f"pst{rep}_{w}_{jp}")
        for t2 in range(2):
            nc.tensor.matmul(
                pst[0:64, t2, :], h_sb[w][:, t2 * 128:t2 * 128 + 64],
                id_sb[:], start=True, stop=True, tile_position=(0, 0),
            )
            nc.tensor.matmul(
                pst[64:128, t2, :],
                h_sb[w][:, t2 * 128 + 64:t2 * 128 + 128],
                id_sb[:], start=True, stop=True, tile_position=(0, 64),
            )
        nc.vector.tensor_copy(stg_tile[:, :, jp % 16, :], pst[:])

    # ---- interleaved dual-window scan ----
    stg_cur = [None] * NWIN
    stg_prev = [None] * NWIN
    for j in range(L):
        jj = j % 16
        mrun = j // RUN
        emit_blocks = sched.get(j, [])
        for w in range(NWIN):
            # xproj filler first so PE stalls on the recurrence are filled
            if emit_blocks:
                emit_xproj(emit_blocks.pop(0))
            # run-buffer rotation (issue run mrun+2 at run boundaries)
            if j % RUN == 0 and j > 0:
                xpf_cur[w] = xpf_nxt[w]
                if mrun + 1 < L // RUN:
                    xpf_nxt[w] = issue_run(w, mrun + 1)
            slot = j % RUN
            xpf = xpf_cur[w]

            # previous iteration's transpose + staging (see emit_pst_stg)
            if j > 0:
                emit_pst_stg(w, j - 1, stg_cur[w])
            if jj == 0:
                stg_prev[w] = stg_cur[w]
                stg_cur[w] = stg_pool.tile(
                    [128, 2, 16, 128], BF16, tag=f"stg{w}",
                    name=f"stg{rep}_{w}_{j // 16}",
                )
                if j > KW:
                    outproj(w, (j - KW) // 16 - 1, stg_prev[w])
            stgw = stg_cur[w]

            def lhs(k):
                if PROBE_CONST_LHS or j == 0:
                    ent = h0_sb
                elif jj == 0:
                    ent = stg_prev[w][:, :, 15, :]
                else:
                    ent = stgw[:, :, jj - 1, :]
                return (ent[:, k, 0:64] if k < 2 else ent[:, k - 2, 64:128])

            # gate PSUM pair tiles: pfi = (f slot0, i slot1), poc = (o, ch).
            # k-outer order: each 64-wide h.T stationary is reused by 4
            # consecutive matmuls on its tile (cheaper LDWEIGHTS).  The xp
            # injection (PE identity matmul) comes LAST (stop=True) so the
            # psum-bank wait of iteration j hides under the k0 stg wait.
            pfi = gps_pool.tile([128, 2, 256], F32, tag=f"g{w}",
                                name=f"pfi{rep}_{w}_{j}")
            poc = gps_pool.tile([128, 2, 256], F32, tag=f"g{w}",
                                name=f"poc{rep}_{w}_{j}")
            for ps, co in ((pfi, 0), (poc, 512)):
                for tp in (0, 64):
                    nc.tensor.matmul(
                        ps[tp:tp + 64, :, :],
                        id_sb[:, tp:tp + 64],
                        xpf[:, slot, co:co + 512],
                        start=True, stop=False,
                        skip_group_check=True,
                        tile_position=(0, tp),
                    )
            for k in range(4):
                lk = lhs(k)
                for tp, hh in ((0, 0), (64, 1)):
                    for ps, sl, g in ((pfi, 1, 1), (pfi, 0, 0),
                                      (poc, 1, 3), (poc, 0, 2)):
                        nc.tensor.matmul(
                            ps[tp:tp + 64, sl, :],
                            lk,
                            whT_sb[:, k, hh * 1024 + g * 256:
                                   hh * 1024 + (g + 1) * 256],
                            start=False, stop=(k == 3),
                            skip_group_check=True,
                            tile_position=(0, tp),
                        )

            a_fi = act_pool.tile([128, 2, 256], F32, tag="a",
                                 name=f"afi{rep}_{w}_{j}")
            nc.scalar.activation(a_fi[:], pfi[:], AF.Sigmoid,
                                 bias=zb[0:128, 0:1])
            a_ch = act_pool.tile([128, 256], F32, tag="a",
                                 name=f"ach{rep}_{w}_{j}")
            nc.scalar.activation(a_ch[:], poc[:, 1, :], AF.Tanh,
                                 bias=zb[0:128, 0:1])
            cm = tmp_pool.tile([128, 256], F32, tag="cm", name=f"cm{rep}_{w}_{j}")
            nc.gpsimd.tensor_mul(cm[:], a_fi[:, 0, :], c_sb[w][:])
            ic = tmp_pool.tile([128, 256], F32, tag="ic", name=f"ic{rep}_{w}_{j}")
            nc.vector.tensor_mul(ic[:], a_fi[:, 1, :], a_ch[:])
            a_o = act_pool.tile([128, 256], F32, tag="a", name=f"ao{rep}_{w}_{j}")
            nc.scalar.activation(a_o[:], poc[:, 0, :], AF.Sigmoid,
                                 bias=zb[0:128, 0:1])
            nc.vector.tensor_add(c_sb[w][:], cm[:], ic[:])
            tc2 = tmp_pool.tile([128, 256], F32, tag="tc", name=f"tc{rep}_{w}_{j}")
            nc.scalar.activation(tc2[:], c_sb[w][:], AF.Tanh, bias=zb[0:128, 0:1])
            nc.vector.tensor_mul(h_sb[w][:], a_o[:], tc2[:])

            if j == KW - 1:
                # masked exact-init merge at emission start
                th = tmp_pool.tile([128, 256], F32, tag="cm", name=f"mh{rep}{w}")
                nc.vector.tensor_mul(th[:], h_sb[w][:], mf_sb[:, w, :])
                nc.vector.tensor_add(h_sb[w][:], th[:], him_sb[:, w, :])
                tcm = tmp_pool.tile([128, 256], F32, tag="ic", name=f"mc{rep}{w}")
                nc.vector.tensor_mul(tcm[:], c_sb[w][:], mf_sb[:, w, :])
                nc.vector.tensor_add(c_sb[w][:], tcm[:], cim_sb[:, w, :])

    for w in range(NWIN):
        emit_pst_stg(w, L - 1, stg_cur[w])
        outproj(w, (L - KW) // 16 - 1, stg_cur[w])


def host_prepare_v4(inputs):
    import ml_dtypes
    bf16 = ml_dtypes.bfloat16
    x = np.asarray(inputs["x"], np.float32)
    Wc = np.concatenate(
        [inputs["Wf_w"], inputs["Wi_w"], inputs["Wo_w"], inputs["Wc_w"]], axis=0
    ).astype(np.float32)
    b = np.concatenate(
        [inputs["Wf_b"], inputs["Wi_b"], inputs["Wo_b"], inputs["Wc_b"]]
    ).astype(np.float32)
    # swizzle gate columns to (half, gate, 256): gate order f, i, o, ch
    perm = np.concatenate([np.arange(g * 512 + h * 256, g * 512 + h * 256 + 256)
                           for h in range(2) for g in range(4)])
    wxT = np.ascontiguousarray(Wc[:, :I].T[:, perm])
    whT = np.ascontiguousarray(Wc[:, I:].T[:, perm]).astype(bf16)
    b = b[perm]
    bxpad = np.zeros((128, G), np.float32)
    bxpad[0] = b
    bxfull = np.broadcast_to(b.astype(bf16), (128, G)).copy()
    e0 = np.zeros((128, 128), np.float32)
    e0[0] = 1.0
    out_w = np.asarray(inputs["out_w"], np.float32)
    out_b = np.asarray(inputs["out_b"], np.float32)
    bh0 = np.asarray(inputs["bh0"], np.float32).reshape(H)
    bc0 = np.asarray(inputs["bc0"], np.float32).reshape(H)

    x_ext = np.zeros((B, T + 2 * KW, I), np.float32)
    x_ext[:, KW:KW + T] = x

    # h0stg[p, t2, r]: unit = (r>=64)*256 + t2*128 + p; state row r%64
    # (0:32 fwd = 0, 32:64 bwd = bh0)
    h0stg = np.zeros((128, 2, 128), np.float32)
    c0fold = np.zeros((128, 256), np.float32)
    for t2 in range(2):
        for half in range(2):
            us = half * 256 + t2 * 128
            h0stg[:, t2, half * 64 + 32:half * 64 + 64] = (
                bh0[us:us + 128].reshape(128, 1).repeat(32, axis=1)
            )
    for half in range(2):
        c0fold[half * 64 + 32:half * 64 + 64, :] = bc0[half * 256:half * 256 + 256]

    shared = {
        "wxT": wxT,
        "bxpad": bxpad,
        "bxfull": bxfull,
        "whT": whT,
        "e0": e0,
        "h0stg": h0stg,
        "c0fold": c0fold,
        "wdTf": np.ascontiguousarray(out_w[:, :H].T).astype(bf16),
        "wdTb": np.ascontiguousarray(out_w[:, H:].T).astype(bf16),
        "ob": out_b.reshape(O, 1),
        "ident": np.eye(128, dtype=np.float32),
    }

    def fold(a):  # [64, 512] state-major -> [128, 256]
        return np.concatenate([a[:, :256], a[:, 256:]], axis=0)

    in_maps = []
    for s in range(NCORES):
        win = x_ext[:, s * 64: s * 64 + WIN]              # [B, WIN, I]
        xtc = np.ascontiguousarray(win.transpose(2, 1, 0).reshape(I, WIN * B))
        m = np.ones((NWIN, 2 * B, H), np.float32)
        ci = np.zeros((NWIN, 2 * B, H), np.float32)
        hi = np.zeros((NWIN, 2 * B, H), np.float32)
        if s == 0:
            m[0, 0:B] = 0.0               # window A fwd: exact zero init
        if s == NCORES - 1:
            m[NWIN - 1, B:2 * B] = 0.0    # window B bwd: exact learned init
            ci[NWIN - 1, B:2 * B] = bc0
            hi[NWIN - 1, B:2 * B] = bh0
        in_maps.append({
            "xt": xtc,
            "mfull": np.stack([fold(m[w]) for w in range(NWIN)]),
            "cim": np.stack([fold(ci[w]) for w in range(NWIN)]),
            "him": np.stack([fold(hi[w]) for w in range(NWIN)]),
            **shared,
        })
    return in_maps


def host_gather_v4(results):
    out = np.zeros((B, T, O), np.float32)
    for s in range(NCORES):
        af = results[s]["outTf"].reshape(O, NWIN, 2, 16, B)
        ab = results[s]["outTb"].reshape(O, NWIN, 2, 16, B)
        for w in range(NWIN):
            for blk in range(2):
                for jj in range(16):
                    tf = 64 * s + 32 * w + 16 * blk + jj
                    tb = 64 * s + 32 * w + 31 - 16 * blk - jj
                    out[:, tf] += af[:, w, blk, jj].T
                    out[:, tb] += ab[:, w, blk, jj].T
    return out


_CACHE = {}


def kernel(**inputs):
    if "nc" not in _CACHE:
        _CACHE["nc"] = build_program_v4()
    nc = _CACHE["nc"]
    in_maps = host_prepare_v4(inputs)
    res = run_bass_kernel_spmd(nc, in_maps, list(range(NCORES)))
    _CACHE["last_exec_time_ns"] = res.exec_time_ns
    return host_gather_v4(res.results)


def run_timed(nc, in_maps, iters=5):
    """Execute the SPMD kernel with device-resident inputs, timing each call."""
    import time as _time
    import jax
    from jax.sharding import Mesh, PartitionSpec, NamedSharding
    from jax.experimental.shard_map import shard_map
    from concourse import bass2jax, mybir as _mb

    bass2jax.install_neuronx_cc_hook()
    n_cores = len(in_maps)

    part_name = nc.partition_id_tensor.name if nc.partition_id_tensor else None
    in_names, out_names, out_avals, zero_outs = [], [], [], []
    for alloc in nc.m.functions[0].allocations:
        if not isinstance(alloc, _mb.MemoryLocationSet):
            continue
        name = alloc.memorylocations[0].name
        if alloc.kind == "ExternalInput":
            if name != part_name:
                in_names.append(name)
        elif alloc.kind == "ExternalOutput":
            out_names.append(name)
            shape = tuple(alloc.tensor_shape)
            dtype = _mb.dt.np(alloc.dtype)
            out_avals.append(jax.core.ShapedArray(shape, dtype))
            zero_outs.append(np.zeros(shape, dtype))
    n_params = len(in_names)
    all_names = in_names + out_names
    if part_name is not None:
        all_names = all_names + [part_name]

    def _body(*args):
        operands = list(args)
        if part_name is not None:
            operands.append(bass2jax.partition_id_tensor())
        outs = bass2jax._bass_exec_p.bind(
            *operands,
            out_avals=tuple(out_avals),
            in_names=tuple(all_names),
            out_names=tuple(out_names),
            lowering_input_output_aliases=(),
            sim_require_finite=True,
            sim_require_nnan=True,
            nc=nc,
        )
        return tuple(outs)

    devices = jax.devices()[:n_cores]
    mesh = Mesh(np.asarray(devices), ("core",))
    spec = PartitionSpec("core")
    nin = n_params + len(out_names)
    fn = jax.jit(
        shard_map(
            _body,
            mesh=mesh,
            in_specs=(spec,) * nin,
            out_specs=(spec,) * len(out_names),
            check_rep=False,
        ),
        keep_unused=True,
    )
    concat_in = [
        np.concatenate([np.asarray(in_maps[c][nm]) for c in range(n_cores)], axis=0)
        for nm in in_names
    ] + [np.zeros((n_cores * z.shape[0], *z.shape[1:]), z.dtype) for z in zero_outs]
    sharding = NamedSharding(mesh, spec)
    dev_in = [jax.device_put(a, sharding) for a in concat_in]
    out = jax.block_until_ready(fn(*dev_in))
    times = []
    for _ in range(iters):
        t0 = _time.perf_counter()
        out = jax.block_until_ready(fn(*dev_in))
        times.append(_time.perf_counter() - t0)
    results = [
        {
            nm: np.asarray(out[i]).reshape(n_cores, *out_avals[i].shape)[c]
            for i, nm in enumerate(out_names)
        }
        for c in range(n_cores)
    ]
    return results, times


# revision 15
# speedup vs baseline: 1.0995x; 1.0995x over previous
"""BiLSTM Trainium2 kernel v4: dual-window interleaved scan + column-tiled PE.

Sharding (8 cores, SPMD): sequence-split; core s owns output steps
[64s, 64s+64) for BOTH directions, processed as TWO interleaved 32-step
windows (A: [64s, 64s+32), B: [64s+32, 64s+64)), each preceded by a KW-step
warmup that converges to the true state through LSTM forget-gate decay.
Interleaving two independent recurrences hides each window's serial
activation/cell-update chain under the other window's matmuls.

Per-iteration j (L = KW + 32 iterations), per window, folded layout
[128, 256] (rows 0:64 = state rows x gate-col half A, 64:128 = half B;
state rows = 32 fwd + 32 bwd):
  gates:  two PSUM tiles [128, 2, 256] hold (f,i) and (o,ch).  Recurrent
      matmuls run COLUMN-TILED: tile T0 (psum partitions 0:64) and T1
      (64:128) execute concurrently in the PE array with the same 64-wide
      h.T stationary and different whT column halves -- 2x matmul
      throughput vs the 128-wide zero-padded fold.  PSUM is prefilled with
      the x-projection by the Pool engine (one [128,2,256] copy per pair).
  acts:   one fused sigmoid over (f,i), tanh(ch), sigmoid(o) on ACT.
  cell:   cm=f*c (Pool), ic=i*ch (DVE), c=cm+ic (DVE), tanh(c) (ACT),
      h=o*tanh(c) (DVE, bf16).
  h.T:    4 column-tiled regular matmuls against a bf16 identity (f32
      PSUM) -> one DVE copy into the bf16 staging block stg[128,2,16,128]
      (cols 0:64 = k-chunks 0/1, 64:128 = chunks 2/3).
  xp:     staged in DRAM bf16 (host-swizzled (half,gate,256) columns) by
      interleaved xproj emission blocks; consumed through [128,4,1024]
      SBUF run buffers loaded 4 iterations at a time (fwd ascending, bwd
      descending via negative-stride DMA), one DMA per quadrant.
  outproj: at each 16-iteration emission block boundary, out.T[128, 512]
      per direction is computed straight from the stg SBUF block (no DRAM
      round-trip) and DMA'd out.
Boundary exactness (fwd t=0 zeros, bwd t=511 learned init) restored by a
masked state merge at the end of warmup, as in v3.
"""

import sys

sys.path.insert(0, "/opt/trn_rl_repo")

import numpy as np
from contextlib import ExitStack

from concourse import bass, bacc, tile, mybir
from concourse.bass_utils import run_bass_kernel_spmd

F32 = mybir.dt.float32
F32R = mybir.dt.float32r
BF16 = mybir.dt.bfloat16
AF = mybir.ActivationFunctionType

B, T, I, H, O = 32, 512, 256, 512, 128
G = 4 * H
NCORES = 8
NWIN = 2
NSW = T // (NCORES * NWIN)      # 32 output steps per window
KW = 16                         # warmup steps
L = KW + NSW                    # 48 scan iterations
WIN = 2 * NSW + 2 * KW          # 96 xp positions per core (window union)
RUN = 4                         # xp run-buffer length (iterations)
PROBE_CONST_LHS = False
NBLK = (WIN * B) // 128         # 24 xproj emission blocks
ROWS_O = NWIN * NSW * B         # 2048 output rows per core


def _r(ap):
    return ap.bitcast(F32R)


def _emission_plan():
    """Block emission order grouped by earliest-need run e(b), and the
    per-iteration schedule: groups 0,1 pre-loop; group g>=2 during
    iterations [4(g-2), 4(g-1))."""
    nrun = L // RUN
    e = {}
    for b_ in range(NBLK):
        cands = []
        if b_ < 12:
            cands.append(b_)           # win A fwd, run b
        if 4 <= b_ <= 15:
            cands.append(15 - b_)      # win A bwd
        if 8 <= b_ <= 19:
            cands.append(b_ - 8)       # win B fwd
        if 12 <= b_ <= 23:
            cands.append(23 - b_)      # win B bwd
        cands = [c for c in cands if 0 <= c < nrun]
        e[b_] = min(cands)
    groups = {}
    for b_, g in e.items():
        groups.setdefault(g, []).append(b_)
    pre = (sorted(groups.get(0, [])) + sorted(groups.get(1, []))
           + sorted(groups.get(2, [])))
    sched = {}  # iter -> list of blocks to emit
    for g in sorted(groups):
        if g < 3:
            continue
        blocks = sorted(groups[g])
        start = 4 * (g - 3)
        for i_, b_ in enumerate(blocks):
            sched.setdefault(start + i_, []).append(b_)
    return pre, sched


def build_program_v4(repeats=1):
    nc = bacc.Bacc(
        "TRN2", target_bir_lowering=False, debug=False, num_devices=NCORES
    )

    xt = nc.dram_tensor("xt", [I, WIN * B], F32, kind="ExternalInput").ap()
    wxT = nc.dram_tensor("wxT", [I, G], F32, kind="ExternalInput").ap()
    bxpad = nc.dram_tensor("bxpad", [128, G], F32, kind="ExternalInput").ap()
    bxfull = nc.dram_tensor("bxfull", [128, G], BF16, kind="ExternalInput").ap()
    whT = nc.dram_tensor("whT", [H, G], BF16, kind="ExternalInput").ap()
    e0 = nc.dram_tensor("e0", [128, 128], F32, kind="ExternalInput").ap()
    h0stg = nc.dram_tensor("h0stg", [128, 2, 128], F32, kind="ExternalInput").ap()
    c0fold = nc.dram_tensor("c0fold", [128, 256], F32, kind="ExternalInput").ap()
    mfull = nc.dram_tensor("mfull", [NWIN, 128, 256], F32, kind="ExternalInput").ap()
    cim = nc.dram_tensor("cim", [NWIN, 128, 256], F32, kind="ExternalInput").ap()
    him = nc.dram_tensor("him", [NWIN, 128, 256], F32, kind="ExternalInput").ap()
    wdTf = nc.dram_tensor("wdTf", [H, O], BF16, kind="ExternalInput").ap()
    wdTb = nc.dram_tensor("wdTb", [H, O], BF16, kind="ExternalInput").ap()
    ob = nc.dram_tensor("ob", [O, 1], F32, kind="ExternalInput").ap()
    ident = nc.dram_tensor("ident", [128, 128], F32, kind="ExternalInput").ap()
    outTf = nc.dram_tensor("outTf", [O, ROWS_O], F32, kind="ExternalOutput").ap()
    outTb = nc.dram_tensor("outTb", [O, ROWS_O], F32, kind="ExternalOutput").ap()

    xp_d = nc.dram_tensor("xp_d", [WIN, B, G], BF16, kind="Internal").ap()

    with tile.TileContext(nc) as tc, ExitStack() as ctx:
        const = ctx.enter_context(tc.tile_pool(name="const", bufs=1))
        gps_pool = ctx.enter_context(tc.tile_pool(name="gps", bufs=2, space="PSUM"))
        xo_pool = ctx.enter_context(tc.tile_pool(name="xo", bufs=2, space="PSUM"))
        pst_pool = ctx.enter_context(tc.tile_pool(name="pst", bufs=2, space="PSUM"))
        xst_pool = ctx.enter_context(tc.tile_pool(name="xst", bufs=3))
        xpf_pool = ctx.enter_context(tc.tile_pool(name="xpf", bufs=3))
        stg_pool = ctx.enter_context(tc.tile_pool(name="stg", bufs=2))
        act_pool = ctx.enter_context(tc.tile_pool(name="act", bufs=8))
        tmp_pool = ctx.enter_context(tc.tile_pool(name="tmp", bufs=3))
        xq_pool = ctx.enter_context(tc.tile_pool(name="xq", bufs=2))
        osb_pool = ctx.enter_context(tc.tile_pool(name="osb", bufs=2))

        # ---- constants ----
        wxT_sb = const.tile([128, 2, G], F32R)
        for c in range(2):
            nc.sync.dma_start(wxT_sb[:, c, :], _r(wxT[c * 128:(c + 1) * 128, :]))
        bxpad_sb = const.tile([128, G], F32R)
        nc.sync.dma_start(bxpad_sb[:], _r(bxpad[:]))
        e0_sb = const.tile([128, 128], F32R)
        nc.sync.dma_start(e0_sb[:], _r(e0[:]))
        bxf_sb = const.tile([128, G], BF16)
        nc.sync.dma_start(bxf_sb[:], bxfull[:])
        whT_sb = const.tile([128, 4, G], BF16)
        for c in range(4):
            nc.sync.dma_start(whT_sb[:, c, :], whT[c * 128:(c + 1) * 128, :])
        h0f = const.tile([128, 2, 128], F32)
        nc.sync.dma_start(h0f[:], h0stg[:])
        h0_sb = const.tile([128, 2, 128], BF16, name="h0ent_sb")
        nc.vector.tensor_copy(h0_sb[:], h0f[:])
        idf = const.tile([128, 128], F32)
        nc.sync.dma_start(idf[:], ident[:])
        id_sb = const.tile([128, 128], BF16)
        nc.vector.tensor_copy(id_sb[:], idf[:])
        wdT_sb = {}
        for d, src in (("f", wdTf), ("b", wdTb)):
            wdT_sb[d] = const.tile([128, 4, O], BF16, name=f"wdT{d}_sb")
            for c in range(4):
                nc.sync.dma_start(wdT_sb[d][:, c, :], src[c * 128:(c + 1) * 128, :])
        ob_sb = const.tile([O, 1], F32)
        nc.sync.dma_start(ob_sb[:], ob[:])
        zb = const.tile([128, 1], F32)
        nc.gpsimd.memset(zb[:], 0.0)
        mf_sb = const.tile([128, NWIN, 256], F32)
        nc.sync.dma_start(mf_sb[:], mfull.transpose([1, 0, 2]))
        cim_sb = const.tile([128, NWIN, 256], F32)
        nc.sync.dma_start(cim_sb[:], cim.transpose([1, 0, 2]))
        him_sb = const.tile([128, NWIN, 256], F32)
        nc.sync.dma_start(him_sb[:], him.transpose([1, 0, 2]))
        c_sb = [const.tile([128, 256], F32, name=f"c{w}_sb") for w in range(NWIN)]
        h_sb = [const.tile([128, 256], BF16, name=f"h{w}_sb") for w in range(NWIN)]
        for w in range(NWIN):
            nc.gpsimd.memset(h_sb[w][:], 0.0)

        for rep in range(repeats):
            _phases_v4(
                nc, tc, rep, xt, xp_d, outTf, outTb, c0fold,
                wxT_sb, bxpad_sb, bxf_sb, e0_sb, whT_sb, h0_sb, id_sb, wdT_sb, ob_sb,
                zb, mf_sb, cim_sb, him_sb, c_sb, h_sb,
                gps_pool, xo_pool, pst_pool, xst_pool, xpf_pool, stg_pool,
                act_pool, tmp_pool, xq_pool, osb_pool,
            )

    nc.compile()
    return nc


def _phases_v4(
    nc, tc, rep, xt, xp_d, outTf, outTb, c0fold,
    wxT_sb, bxpad_sb, bxf_sb, e0_sb, whT_sb, h0_sb, id_sb, wdT_sb, ob_sb,
    zb, mf_sb, cim_sb, him_sb, c_sb, h_sb,
    gps_pool, xo_pool, pst_pool, xst_pool, xpf_pool, stg_pool,
    act_pool, tmp_pool, xq_pool, osb_pool,
):
    # per-repeat state init (fwd rows zero, bwd rows learned cell init)
    for w in range(NWIN):
        nc.sync.dma_start(c_sb[w][:], c0fold[:])

    def emit_xproj(b_):
        """xproj emission for block b_ (positions 4b..4b+3): 4 column-tiled
        psums [128, 512] -> one bf16 xq tile -> one DMA into xp_d."""
        xst = xst_pool.tile([128, 2, 128], F32R, tag="xst", name=f"xst{rep}_{b_}")
        for c in range(2):
            nc.sync.dma_start(
                xst[:, c, :],
                _r(xt[c * 128:(c + 1) * 128, b_ * 128:(b_ + 1) * 128]),
            )
        xq = xq_pool.tile([128, G], BF16, tag="xq", name=f"xq{rep}_{b_}")
        for s in range(4):
            # f32r matmuls cannot be column-tiled (invalid ISA); these run
            # untiled (128x128 mode) -- the PE drains on the mode switch.
            # Bias is folded into the DVE evacuation add (bxfull).
            ps = xo_pool.tile([128, 512], F32, tag="xo", name=f"xps{rep}_{b_}_{s}")
            for c in range(2):
                nc.tensor.matmul(
                    ps[:],
                    xst[:, c, :],
                    wxT_sb[:, c, s * 512:(s + 1) * 512],
                    start=(c == 0), stop=(c == 1),
                )
            nc.vector.tensor_add(
                xq[:, s * 512:(s + 1) * 512], ps[:],
                bxf_sb[:, s * 512:(s + 1) * 512],
            )
        nc.scalar.dma_start(
            xp_d.flatten_outer_dims()[b_ * 128:(b_ + 1) * 128, :], xq[:]
        )

    def issue_run(w, m):
        """Load xp run m for window w into an xpf tile: fwd positions
        32w+4m.. ascending on partitions 0:32/64:96, bwd positions
        63+32w-4m.. descending on 32:64/96:128."""
        xpf = xpf_pool.tile([128, RUN, 1024], BF16, tag=f"xpf{w}",
                            name=f"xpf{rep}_{w}_{m}")
        fb = 32 * w + RUN * m
        bb = 63 + 32 * w - RUN * m - (RUN - 1)
        nc.sync.dma_start(
            xpf[0:32, :, :], xp_d[fb:fb + RUN, :, 0:1024].transpose([1, 0, 2])
        )
        nc.sync.dma_start(
            xpf[64:96, :, :],
            xp_d[fb:fb + RUN, :, 1024:2048].transpose([1, 0, 2]),
        )
        nc.sync.dma_start(
            xpf[32:64, :, :],
            xp_d[bb:bb + RUN, :, 0:1024][::-1].transpose([1, 0, 2]),
        )
        nc.sync.dma_start(
            xpf[96:128, :, :],
            xp_d[bb:bb + RUN, :, 1024:2048][::-1].transpose([1, 0, 2]),
        )
        return xpf

    pre, sched = _emission_plan()
    for b_ in pre:
        emit_xproj(b_)

    # lead-2 prefetch, window issues staggered by 2 iterations (W0 at
    # j%4==0, W1 at j%4==2).  Early-issued loads safely wait on the
    # emission stores' DMA semaphores.
    xpf_tiles = [{0: issue_run(w, 0), 1: issue_run(w, 1)} for w in range(NWIN)]

    def outproj(w, blk, stgw):
        """Output projection for emission block blk of window w, straight
        from the stg SBUF block (fwd rows = cols 0:32/64:96 per t2)."""
        for d, outdst, bias in (("f", outTf, ob_sb[0:O, 0:1]),
                                ("b", outTb, zb[0:O, 0:1])):
            ro = 32 if d == "b" else 0
            ps = xo_pool.tile([128, 512], F32, tag="xo",
                              name=f"ops{rep}_{w}_{blk}{d}")
            for k in range(4):
                rhs = (stgw[:, k, :, ro:ro + 32] if k < 2
                       else stgw[:, k - 2, :, 64 + ro:96 + ro])
                for tp in (0, 64):
                    nc.tensor.matmul(
                        ps[tp:tp + 64, :],
                        wdT_sb[d][:, k, tp:tp + 64],
                        rhs,
                        start=(k == 0), stop=(k == 3),
                        tile_position=(0, tp),
                    )
            osb = osb_pool.tile([O, 512], F32, tag="osb",
                                name=f"osb{rep}_{w}_{blk}{d}")
            nc.scalar.activation(osb[:], ps[:], AF.Identity, bias=bias)
            nc.sync.dma_start(
                outdst[:, w * 1024 + blk * 512:w * 1024 + (blk + 1) * 512],
                osb[:],
            )

    def emit_pst_stg(w, jp, stg_tile):
        """h(jp).T via 4 column-tiled regular matmuls (f32 psum) + 1 DVE copy
        into stg slot jp%16.  Issued at the TOP of the window's next
        iteration block so it never head-blocks the other window's ready
        matmuls in the in-order PE queue."""
        pst = pst_pool.tile([128, 2, 128], F32, tag="pst",
                            name=f"pst{rep}_{w}_{jp}")
        for t2 in range(2):
            nc.tensor.matmul(
                pst[0:64, t2, :], h_sb[w][:, t2 * 128:t2 * 128 + 64],
                id_sb[:], start=True, stop=True, tile_position=(0, 0),
            )
            nc.tensor.matmul(
                pst[64:128, t2, :],
                h_sb[w][:, t2 * 128 + 64:t2 * 128 + 128],
                id_sb[:], start=True, stop=True, tile_position=(0, 64),
            )
        nc.vector.tensor_copy(stg_tile[:, :, jp % 16, :], pst[:])

    # ---- interleaved dual-window scan ----
    stg_cur = [None] * NWIN
    stg_prev = [None] * NWIN
    for j in range(L):
        jj = j % 16
        mrun = j // RUN
        emit_blocks = sched.get(j, [])
        for w in range(NWIN):
            # xproj filler first so PE stalls on the recurrence are filled
            if emit_blocks:
                emit_xproj(emit_blocks.pop(0))
            # staggered lead-2 run prefetch
            if j % RUN == 2 * w and mrun + 2 < L // RUN:
                xpf_tiles[w][mrun + 2] = issue_run(w, mrun + 2)
            slot = j % RUN
            xpf = xpf_tiles[w][mrun]
            xpf_tiles[w].pop(mrun - 1, None)

            # previous iteration's transpose + staging (see emit_pst_stg)
            if j > 0:
                emit_pst_stg(w, j - 1, stg_cur[w])
            if jj == 0:
                stg_prev[w] = stg_cur[w]
                stg_cur[w] = stg_pool.tile(
                    [128, 2, 16, 128], BF16, tag=f"stg{w}",
                    name=f"stg{rep}_{w}_{j // 16}",
                )
                if j > KW:
                    outproj(w, (j - KW) // 16 - 1, stg_prev[w])
            stgw = stg_cur[w]

            def lhs(k):
                if PROBE_CONST_LHS or j == 0:
                    ent = h0_sb
                elif jj == 0:
                    ent = stg_prev[w][:, :, 15, :]
                else:
                    ent = stgw[:, :, jj - 1, :]
                return (ent[:, k, 0:64] if k < 2 else ent[:, k - 2, 64:128])

            # gate PSUM pair tiles: pfi = (f slot0, i slot1), poc = (o, ch).
            # k-outer order: each 64-wide h.T stationary is reused by 4
            # consecutive matmuls on its tile (cheaper LDWEIGHTS).  The xp
            # injection (PE identity matmul) comes LAST (stop=True) so the
            # psum-bank wait of iteration j hides under the k0 stg wait.
            pfi = gps_pool.tile([128, 2, 256], F32, tag=f"g{w}",
                                name=f"pfi{rep}_{w}_{j}")
            poc = gps_pool.tile([128, 2, 256], F32, tag=f"g{w}",
                                name=f"poc{rep}_{w}_{j}")
            for ps, co in ((pfi, 0), (poc, 512)):
                for tp in (0, 64):
                    nc.tensor.matmul(
                        ps[tp:tp + 64, :, :],
                        id_sb[:, tp:tp + 64],
                        xpf[:, slot, co:co + 512],
                        start=True, stop=False,
                        skip_group_check=True,
                        tile_position=(0, tp),
                    )
            for k in range(4):
                lk = lhs(k)
                for tp, hh in ((0, 0), (64, 1)):
                    for ps, sl, g in ((pfi, 1, 1), (pfi, 0, 0),
                                      (poc, 1, 3), (poc, 0, 2)):
                        nc.tensor.matmul(
                            ps[tp:tp + 64, sl, :],
                            lk,
                            whT_sb[:, k, hh * 1024 + g * 256:
                                   hh * 1024 + (g + 1) * 256],
                            start=False, stop=(k == 3),
                            skip_group_check=True,
                            tile_position=(0, tp),
                        )

            a_fi = act_pool.tile([128, 2, 256], F32, tag="a",
                                 name=f"afi{rep}_{w}_{j}")
            nc.scalar.activation(a_fi[:], pfi[:], AF.Sigmoid,
                                 bias=zb[0:128, 0:1])
            a_ch = act_pool.tile([128, 256], F32, tag="a",
                                 name=f"ach{rep}_{w}_{j}")
            nc.scalar.activation(a_ch[:], poc[:, 1, :], AF.Tanh,
                                 bias=zb[0:128, 0:1])
            cm = tmp_pool.tile([128, 256], F32, tag="cm", name=f"cm{rep}_{w}_{j}")
            nc.gpsimd.tensor_mul(cm[:], a_fi[:, 0, :], c_sb[w][:])
            ic = tmp_pool.tile([128, 256], F32, tag="ic", name=f"ic{rep}_{w}_{j}")
            nc.gpsimd.tensor_mul(ic[:], a_fi[:, 1, :], a_ch[:])
            a_o = act_pool.tile([128, 256], F32, tag="a", name=f"ao{rep}_{w}_{j}")
            nc.scalar.activation(a_o[:], poc[:, 0, :], AF.Sigmoid,
                                 bias=zb[0:128, 0:1])
            nc.gpsimd.tensor_add(c_sb[w][:], cm[:], ic[:])
            tc2 = tmp_pool.tile([128, 256], F32, tag="tc", name=f"tc{rep}_{w}_{j}")
            nc.scalar.activation(tc2[:], c_sb[w][:], AF.Tanh, bias=zb[0:128, 0:1])
            nc.gpsimd.tensor_mul(h_sb[w][:], a_o[:], tc2[:])

            if j == KW - 1:
                # masked exact-init merge at emission start
                th = tmp_pool.tile([128, 256], F32, tag="cm", name=f"mh{rep}{w}")
                nc.vector.tensor_mul(th[:], h_sb[w][:], mf_sb[:, w, :])
                nc.vector.tensor_add(h_sb[w][:], th[:], him_sb[:, w, :])
                tcm = tmp_pool.tile([128, 256], F32, tag="ic", name=f"mc{rep}{w}")
                nc.vector.tensor_mul(tcm[:], c_sb[w][:], mf_sb[:, w, :])
                nc.vector.tensor_add(c_sb[w][:], tcm[:], cim_sb[:, w, :])

    for w in range(NWIN):
        emit_pst_stg(w, L - 1, stg_cur[w])
        outproj(w, (L - KW) // 16 - 1, stg_cur[w])


def host_prepare_v4(inputs):
    import ml_dtypes
    bf16 = ml_dtypes.bfloat16
    x = np.asarray(inputs["x"], np.float32)
    Wc = np.concatenate(
        [inputs["Wf_w"], inputs["Wi_w"], inputs["Wo_w"], inputs["Wc_w"]], axis=0
    ).astype(np.float32)
    b = np.concatenate(
        [inputs["Wf_b"], inputs["Wi_b"], inputs["Wo_b"], inputs["Wc_b"]]
    ).astype(np.float32)
    # swizzle gate columns to (half, gate, 256): gate order f, i, o, ch
    perm = np.concatenate([np.arange(g * 512 + h * 256, g * 512 + h * 256 + 256)
                           for h in range(2) for g in range(4)])
    wxT = np.ascontiguousarray(Wc[:, :I].T[:, perm])
    whT = np.ascontiguousarray(Wc[:, I:].T[:, perm]).astype(bf16)
    b = b[perm]
    bxpad = np.zeros((128, G), np.float32)
    bxpad[0] = b
    bxfull = np.broadcast_to(b.astype(bf16), (128, G)).copy()
    e0 = np.zeros((128, 128), np.float32)
    e0[0] = 1.0
    out_w = np.asarray(inputs["out_w"], np.float32)
    out_b = np.asarray(inputs["out_b"], np.float32)
    bh0 = np.asarray(inputs["bh0"], np.float32).reshape(H)
    bc0 = np.asarray(inputs["bc0"], np.float32).reshape(H)

    x_ext = np.zeros((B, T + 2 * KW, I), np.float32)
    x_ext[:, KW:KW + T] = x

    # h0stg[p, t2, r]: unit = (r>=64)*256 + t2*128 + p; state row r%64
    # (0:32 fwd = 0, 32:64 bwd = bh0)
    h0stg = np.zeros((128, 2, 128), np.float32)
    c0fold = np.zeros((128, 256), np.float32)
    for t2 in range(2):
        for half in range(2):
            us = half * 256 + t2 * 128
            h0stg[:, t2, half * 64 + 32:half * 64 + 64] = (
                bh0[us:us + 128].reshape(128, 1).repeat(32, axis=1)
            )
    for half in range(2):
        c0fold[half * 64 + 32:half * 64 + 64, :] = bc0[half * 256:half * 256 + 256]

    shared = {
        "wxT": wxT,
        "bxpad": bxpad,
        "bxfull": bxfull,
        "whT": whT,
        "e0": e0,
        "h0stg": h0stg,
        "c0fold": c0fold,
        "wdTf": np.ascontiguousarray(out_w[:, :H].T).astype(bf16),
        "wdTb": np.ascontiguousarray(out_w[:, H:].T).astype(bf16),
        "ob": out_b.reshape(O, 1),
        "ident": np.eye(128, dtype=np.float32),
    }

    def fold(a):  # [64, 512] state-major -> [128, 256]
        return np.concatenate([a[:, :256], a[:, 256:]], axis=0)

    in_maps = []
    for s in range(NCORES):
        win = x_ext[:, s * 64: s * 64 + WIN]              # [B, WIN, I]
        xtc = np.ascontiguousarray(win.transpose(2, 1, 0).reshape(I, WIN * B))
        m = np.ones((NWIN, 2 * B, H), np.float32)
        ci = np.zeros((NWIN, 2 * B, H), np.float32)
        hi = np.zeros((NWIN, 2 * B, H), np.float32)
        if s == 0:
            m[0, 0:B] = 0.0               # window A fwd: exact zero init
        if s == NCORES - 1:
            m[NWIN - 1, B:2 * B] = 0.0    # window B bwd: exact learned init
            ci[NWIN - 1, B:2 * B] = bc0
            hi[NWIN - 1, B:2 * B] = bh0
        in_maps.append({
            "xt": xtc,
            "mfull": np.stack([fold(m[w]) for w in range(NWIN)]),
            "cim": np.stack([fold(ci[w]) for w in range(NWIN)]),
            "him": np.stack([fold(hi[w]) for w in range(NWIN)]),
            **shared,
        })
    return in_maps


def host_gather_v4(results):
    out = np.zeros((B, T, O), np.float32)
    for s in range(NCORES):
        af = results[s]["outTf"].reshape(O, NWIN, 2, 16, B)
        ab = results[s]["outTb"].reshape(O, NWIN, 2, 16, B)
        for w in range(NWIN):
            for blk in range(2):
                for jj in range(16):
                    tf = 64 * s + 32 * w + 16 * blk + jj
                    tb = 64 * s + 32 * w + 31 - 16 * blk - jj
                    out[:, tf] += af[:, w, blk, jj].T
                    out[:, tb] += ab[:, w, blk, jj].T
    return out


_CACHE = {}


def kernel(**inputs):
    if "nc" not in _CACHE:
        _CACHE["nc"] = build_program_v4()
    nc = _CACHE["nc"]
    in_maps = host_prepare_v4(inputs)
    res = run_bass_kernel_spmd(nc, in_maps, list(range(NCORES)))
    _CACHE["last_exec_time_ns"] = res.exec_time_ns
    return host_gather_v4(res.results)


def run_timed(nc, in_maps, iters=5):
    """Execute the SPMD kernel with device-resident inputs, timing each call."""
    import time as _time
    import jax
    from jax.sharding import Mesh, PartitionSpec, NamedSharding
    from jax.experimental.shard_map import shard_map
    from concourse import bass2jax, mybir as _mb

    bass2jax.install_neuronx_cc_hook()
    n_cores = len(in_maps)

    part_name = nc.partition_id_tensor.name if nc.partition_id_tensor else None
    in_names, out_names, out_avals, zero_outs = [], [], [], []
    for alloc in nc.m.functions[0].allocations:
        if not isinstance(alloc, _mb.MemoryLocationSet):
            continue
        name = alloc.memorylocations[0].name
        if alloc.kind == "ExternalInput":
            if name != part_name:
                in_names.append(name)
        elif alloc.kind == "ExternalOutput":
            out_names.append(name)
            shape = tuple(alloc.tensor_shape)
            dtype = _mb.dt.np(alloc.dtype)
            out_avals.append(jax.core.ShapedArray(shape, dtype))
            zero_outs.append(np.zeros(shape, dtype))
    n_params = len(in_names)
    all_names = in_names + out_names
    if part_name is not None:
        all_names = all_names + [part_name]

    def _body(*args):
        operands = list(args)
        if part_name is not None:
            operands.append(bass2jax.partition_id_tensor())
        outs = bass2jax._bass_exec_p.bind(
            *operands,
            out_avals=tuple(out_avals),
            in_names=tuple(all_names),
            out_names=tuple(out_names),
            lowering_input_output_aliases=(),
            sim_require_finite=True,
            sim_require_nnan=True,
            nc=nc,
        )
        return tuple(outs)

    devices = jax.devices()[:n_cores]
    mesh = Mesh(np.asarray(devices), ("core",))
    spec = PartitionSpec("core")
    nin = n_params + len(out_names)
    fn = jax.jit(
        shard_map(
            _body,
            mesh=mesh,
            in_specs=(spec,) * nin,
            out_specs=(spec,) * len(out_names),
            check_rep=False,
        ),
        keep_unused=True,
    )
    concat_in = [
        np.concatenate([np.asarray(in_maps[c][nm]) for c in range(n_cores)], axis=0)
        for nm in in_names
    ] + [np.zeros((n_cores * z.shape[0], *z.shape[1:]), z.dtype) for z in zero_outs]
    sharding = NamedSharding(mesh, spec)
    dev_in = [jax.device_put(a, sharding) for a in concat_in]
    out = jax.block_until_ready(fn(*dev_in))
    times = []
    for _ in range(iters):
        t0 = _time.perf_counter()
        out = jax.block_until_ready(fn(*dev_in))
        times.append(_time.perf_counter() - t0)
    results = [
        {
            nm: np.asarray(out[i]).reshape(n_cores, *out_avals[i].shape)[c]
            for i, nm in enumerate(out_names)
        }
        for c in range(n_cores)
    ]
    return results, times
